# revision 27
# baseline (speedup 1.0000x reference)
"""Trainium2 Bass kernel: top-2 MoE routing (E=16, D=H=2048), 8 NeuronCores.

v3 strategy (memory regime -- minimize dynamic HBM bytes + latency):
  * Host pre-quantizes weights to fp8-e3m4 (4-bit mantissa): W1's scale is
    folded into x, W2's into the normalized gates -- cutting the per-core
    dynamic weight stream from 8MB fp32 to ~2.1MB (360 GB/s floor ~6us).
  * Gating runs twice on PE: a bf16 fast path feeding DVE max/max_index
    (top-2 indices in 2 ops) to start the weight DMAs ASAP, and an fp32
    replay (off the critical path) for exact softmax gate values.
  * Expert weights are fetched with register-indexed DMAs (idx -> SP regs);
    the last chunk is a single 32KB column so the post-stream tail is one
    Ldweights.
  * b1 and b2 are loaded for ALL experts statically (idle prefix bus, expert
    dim on partitions) and folded into the PSUM accumulations via K=16
    matmuls: b1 against the one-hot expert masks (is_equal of a partition
    iota vs the broadcast index), b2 against gsel (the same masks scaled by
    the gates) -- keeping the dynamic weight stream 100% gapless.
  * Layer 1: h = tanh(W1[e] @ x + b1[e]) as accumulating [128,128]x[128,1]
    matmuls; h is scaled by tkg_e/s2 so layer 2 accumulates gate-weighted
    outputs directly. Tail: one DVE add (eo1 + early-evacuated eo0) and a
    64B-row output DMA.
  * Each core owns rows [c*256,(c+1)*256) of every expert's W1 and the
    matching contraction slice of W2; host sums the 8 partial outputs.
"""

import numpy as np

try:  # make concourse importable in bare environments
    import concourse.bacc  # noqa: F401
except ImportError:  # pragma: no cover
    import sys

    sys.path.insert(0, "/opt/trn_rl_repo")

import ml_dtypes

E, D, H = 16, 2048, 2048
NCORES = 8
P = 128
RS = H // NCORES  # 256 rows of each expert held per core
NCH = RS // P  # 2 partition-chunks per 256 rows
DC = D // P  # 16 contraction chunks for layer 1
OC = H // P  # 16 output chunks for layer 2

S1 = 100.0  # W1 quant scale (folded into xl1)
S2 = 100.0  # W2 quant scale (folded into tkg)
W2_BF16 = False  # True: W2 in bf16 (safer numerics, +3us stream)

F1 = DC * RS  # 4096 cols of w1q (pure weights; b1 loads separately)
F2 = NCH * H  # 4096 cols of w2q
F2_SPLIT = (NCH - 1) * H + (OC - 1) * P  # 3968: tiny last chunk (1 col)

_BUILT = None


def _build():
    """Build + compile the Bass program once. Returns (nc, input_names)."""
    global _BUILT
    if _BUILT is not None:
        return _BUILT

    import concourse.bacc as bacc
    import concourse.bass as bass
    import concourse.tile as tile
    from concourse import mybir

    f32 = mybir.dt.float32
    bf16 = mybir.dt.bfloat16
    f8 = mybir.dt.float8e3
    u32 = mybir.dt.uint32
    w2dt = bf16 if W2_BF16 else f8
    AX = mybir.AxisListType.X
    OP = mybir.AluOpType
    ACTF = mybir.ActivationFunctionType

    nc = bacc.Bacc(
        "TRN2", target_bir_lowering=False, debug=False, num_devices=NCORES
    )

    # ----- I/O --------------------------------------------------------------
    # gpbf: [x_pd(16) | WgT(256) | bg(16 on row0)] in bf16
    gpbf_d = nc.dram_tensor("gpbf", [P, 288], bf16, kind="ExternalInput")
    # gpf: same in f32, plus col 288 = partition iota (rows 0..15) and
    # cols 289..416 = ones on row 0 (broadcast-matmul lhsT)
    gpf_d = nc.dram_tensor("gpf", [P, 417], f32, kind="ExternalInput")
    # xl1: x/S1 on partitions by contraction chunk
    xl1_d = nc.dram_tensor("xl1", [P, DC], bf16, kind="ExternalInput")
    # w1q: per expert [128p, dc*256+j] = W1[e, c*256+j, dc*128+p]*S1 (e3m4)
    w1q_d = nc.dram_tensor("w1q", [E, P, F1], f8, kind="ExternalInput")
    # b1pk: [e, j] = b1[e, c*256+j] (expert dim on partitions)
    b1pk_d = nc.dram_tensor("b1pk", [E, RS], f32, kind="ExternalInput")
    # w2q: per expert [128p, ic*2048+o] = W2[e, o, c*256+ic*128+p]*S2
    w2q_d = nc.dram_tensor("w2q", [E, P, F2], w2dt, kind="ExternalInput")
    # b2pk: [e, o] = b2[e, o] * S2 / NCORES (expert dim on partitions)
    b2pk_d = nc.dram_tensor("b2pk", [E, H], f32, kind="ExternalInput")
    out_d = nc.dram_tensor("out", [P, OC], f32, kind="ExternalOutput")

    in_names = ["gpbf", "gpf", "xl1", "w1q", "w2q", "b1pk", "b2pk"]

    with tile.TileContext(nc) as tc:
        with (
            tc.tile_pool(name="sb", bufs=1) as sb,
            tc.tile_pool(name="ps", bufs=1, space="PSUM") as ps,
        ):
            # ----- static loads --------------------------------------------
            gp_bf = sb.tile([P, 288], bf16, tag="gp_bf")
            nc.sync.dma_start(gp_bf[:], gpbf_d.ap())  # SP: critical path
            gp_f = sb.tile([P, 417], f32, tag="gp_f")
            nc.scalar.dma_start(gp_f[:], gpf_d.ap())
            xl1 = sb.tile([P, DC], bf16, tag="xl1")
            nc.scalar.dma_start(xl1[:], xl1_d.ap())
            # all experts' b1/scaled-b2, expert dim on partitions (static)
            b1pk = sb.tile([E, RS], f32, tag="b1pk")
            nc.scalar.dma_start(b1pk[:], b1pk_d.ap())
            b2pk = sb.tile([E, H], f32, tag="b2pk")
            nc.scalar.dma_start(b2pk[:], b2pk_d.ap())

            # program-preamble const tensors (memset before the entry barrier)
            one_bf = nc.const_aps.aps[(bf16, 1.0)][0:1, 0:1]
            one_f = nc.const_aps.aps[(f32, 1.0)][0:1, 0:1]
            ones_col = gp_f[0:1, 289 : 289 + P]

            # ----- gating fast path (bf16) on PE ---------------------------
            lg_bf = ps.tile([1, E], f32, tag="lg_bf")
            for dc in range(DC):
                nc.tensor.matmul(
                    out=lg_bf[:],
                    lhsT=gp_bf[:, dc : dc + 1],
                    rhs=gp_bf[:, 16 + dc * E : 16 + (dc + 1) * E],
                    start=(dc == 0),
                    stop=False,
                )
            nc.tensor.matmul(
                out=lg_bf[:], lhsT=one_bf, rhs=gp_bf[0:1, 272:288],
                start=False, stop=True,
            )

            # top-2 indices in two DVE ops
            vmax_bf = sb.tile([1, 8], f32, tag="vmax_bf")
            nc.vector.max(vmax_bf[:], lg_bf[:])
            vidx = sb.tile([1, 8], u32, tag="vidx")
            nc.vector.max_index(vidx[:], vmax_bf[:], lg_bf[:])

            # ----- idx -> SP registers -> dynamic weight DMAs ---------------
            sv = []
            for k in range(2):
                reg = nc.sync.alloc_register(f"idx_sp{k}")
                nc.sync.reg_load(reg, vidx[0:1, k : k + 1])
                sv.append(nc.snap(reg, donate=True, min_val=0, max_val=E - 1))

            w1t = [
                sb.tile([P, F1], f8, tag=f"w1t{k}", name=f"w1t{k}")
                for k in range(2)
            ]
            w2t = [
                sb.tile([P, F2], w2dt, tag=f"w2t{k}", name=f"w2t{k}")
                for k in range(2)
            ]
            for k in range(2):
                nc.sync.dma_start(
                    w1t[k][:],
                    w1q_d.ap()[bass.ds(sv[k], 1), :, :].rearrange(
                        "o p f -> p (o f)"
                    ),
                )
            nc.sync.dma_start(
                w2t[0][:],
                w2q_d.ap()[bass.ds(sv[0], 1), :, :].rearrange(
                    "o p f -> p (o f)"
                ),
            )
            # e1's W2 split so the last DMA chunk (oc 14..15 of ic 1) is small
            nc.sync.dma_start(
                w2t[1][:, 0:F2_SPLIT],
                w2q_d.ap()[bass.ds(sv[1], 1), :, 0:F2_SPLIT].rearrange(
                    "o p f -> p (o f)"
                ),
            )
            nc.sync.dma_start(
                w2t[1][:, F2_SPLIT:F2],
                w2q_d.ap()[bass.ds(sv[1], 1), :, F2_SPLIT:F2].rearrange(
                    "o p f -> p (o f)"
                ),
            )

            # ----- gating slow path (fp32) for exact gate values ------------
            lg_f = ps.tile([1, E], f32, tag="lg_f")
            for dc in range(DC):
                nc.tensor.matmul(
                    out=lg_f[:],
                    lhsT=gp_f[:, dc : dc + 1],
                    rhs=gp_f[:, 16 + dc * E : 16 + (dc + 1) * E],
                    start=(dc == 0),
                    stop=False,
                )
            nc.tensor.matmul(
                out=lg_f[:], lhsT=one_f, rhs=gp_f[0:1, 272:288],
                start=False, stop=True,
            )

            # tkg'_k = tkg_k / S2, with tkg = top2(softmax)/(sum+1e-6)
            vmax_f = sb.tile([1, 8], f32, tag="vmax_f")
            nc.vector.max(vmax_f[:], lg_f[:])
            negm1 = sb.tile([1, 1], f32, tag="negm1")
            nc.vector.tensor_scalar_mul(negm1[:], vmax_f[0:1, 0:1], -1.0)
            esb = sb.tile([1, E], f32, tag="esb")
            nc.scalar.activation(esb[:], lg_f[:], ACTF.Exp, bias=negm1[:])
            e2x = sb.tile([1, 1], f32, tag="e2x")
            nc.scalar.activation(
                e2x[:], vmax_f[0:1, 1:2], ACTF.Exp, bias=negm1[:]
            )
            ssum = sb.tile([1, 1], f32, tag="ssum")
            nc.vector.tensor_reduce(ssum[:], esb[:], axis=AX, op=OP.add)
            # den2 = S2 * (1 + e2x + 1e-6*ssum)
            den2 = sb.tile([1, 1], f32, tag="den2")
            nc.vector.tensor_scalar(
                den2[:], ssum[:], 1e-6 * S2, S2, OP.mult, OP.add
            )
            e2xs = sb.tile([1, 1], f32, tag="e2xs")
            nc.vector.tensor_scalar_mul(e2xs[:], e2x[:], S2)
            nc.vector.tensor_add(den2[:], den2[:], e2xs[:])
            tkgp = [
                sb.tile([1, 1], f32, tag=f"tkgp{k}", name=f"tkgp{k}")
                for k in range(2)
            ]
            nc.vector.reciprocal(tkgp[0][:], den2[:])
            nc.vector.tensor_mul(tkgp[1][:], e2x[:], tkgp[0][:])

            # broadcast tkg' to all partitions via K=1 matmuls
            tkgrep = ps.tile([P, 2], f32, tag="tkgrep")
            for k in range(2):
                nc.tensor.matmul(
                    out=tkgrep[:, k : k + 1],
                    lhsT=ones_col,
                    rhs=tkgp[k][:],
                    start=True,
                    stop=True,
                )

            # gsel[e] = sum_k tkg'_k * [e == idx_k]  (16-partition one-hot mix
            # vector; folds the b2 bias into the layer-2 PSUM accumulation)
            idxf = sb.tile([1, 2], f32, tag="idxf")
            nc.vector.tensor_copy(idxf[:], vidx[0:1, 0:2])
            idx_ps = ps.tile([E, 2], f32, tag="idx_ps")
            for k in range(2):
                nc.tensor.matmul(
                    out=idx_ps[:, k : k + 1],
                    lhsT=ones_col[0:1, 0:E],
                    rhs=idxf[0:1, k : k + 1],
                    start=True,
                    stop=True,
                )
            mk = [
                sb.tile([E, 1], f32, tag=f"mk{k}", name=f"mk{k}")
                for k in range(2)
            ]
            gs = [
                sb.tile([E, 1], f32, tag=f"gs{k}", name=f"gs{k}")
                for k in range(2)
            ]
            for k in range(2):
                nc.vector.tensor_tensor(
                    out=mk[k][:],
                    in0=gp_f[0:E, 288:289],
                    in1=idx_ps[:, k : k + 1],
                    op=OP.is_equal,
                )
                nc.vector.tensor_scalar(
                    gs[k][:], mk[k][:], tkgrep[0:E, k : k + 1], None, OP.mult
                )
            gsel = sb.tile([E, 1], f32, tag="gsel")
            nc.vector.tensor_add(gsel[:], gs[0][:], gs[1][:])

            # ----- layer 1: h = tanh(W1[e] @ x + b1[e]), scaled by tkg' -----
            h_ps = [
                ps.tile([P, NCH], f32, tag=f"hps{k}", name=f"hps{k}")
                for k in range(2)
            ]
            hs = [
                sb.tile([P, NCH], bf16, tag=f"hs{k}", name=f"hs{k}")
                for k in range(2)
            ]
            for k in range(2):
                for rc in range(NCH):
                    for dc in range(DC):
                        nc.tensor.matmul(
                            out=h_ps[k][:, rc : rc + 1],
                            lhsT=w1t[k][
                                :, dc * RS + rc * P : dc * RS + (rc + 1) * P
                            ],
                            rhs=xl1[:, dc : dc + 1],
                            start=(dc == 0),
                            stop=False,
                        )
                    # b1[e] via K=16 matmul against the one-hot mask
                    nc.tensor.matmul(
                        out=h_ps[k][:, rc : rc + 1],
                        lhsT=b1pk[:, rc * P : (rc + 1) * P],
                        rhs=mk[k][:],
                        start=False,
                        stop=True,
                    )
                for rc in range(NCH):
                    nc.scalar.activation(
                        hs[k][:, rc : rc + 1],
                        h_ps[k][:, rc : rc + 1],
                        ACTF.Tanh,
                    )
                nc.vector.tensor_scalar(
                    hs[k][:], hs[k][:], tkgrep[:, k : k + 1], None, OP.mult
                )

            # ----- layer 2: eo_k = W2[e_k] @ hs_k (gate-weighted); the b2
            # bias term (gsel-weighted mix over all experts) accumulates
            # into expert 0's PSUM between its ic chunks -------------------
            eo_ps = [
                ps.tile([P, OC], f32, tag=f"eops{k}", name=f"eops{k}")
                for k in range(2)
            ]
            for k in range(2):
                for oc in range(OC):
                    nc.tensor.matmul(
                        out=eo_ps[k][:, oc : oc + 1],
                        lhsT=w2t[k][:, oc * P : (oc + 1) * P],
                        rhs=hs[k][:, 0:1],
                        start=True,
                        stop=False,
                    )
                    if k == 0:
                        nc.tensor.matmul(
                            out=eo_ps[k][:, oc : oc + 1],
                            lhsT=b2pk[:, oc * P : (oc + 1) * P],
                            rhs=gsel[:],
                            start=False,
                            stop=False,
                        )
                    nc.tensor.matmul(
                        out=eo_ps[k][:, oc : oc + 1],
                        lhsT=w2t[k][:, H + oc * P : H + (oc + 1) * P],
                        rhs=hs[k][:, 1:2],
                        start=False,
                        stop=True,
                    )

            # ----- combine: res = (eo0 + b2mix) + eo1 -----------------------
            ea0 = sb.tile([P, OC], f32, tag="ea0")
            nc.vector.tensor_copy(ea0[:], eo_ps[0][:])
            res = sb.tile([P, OC], f32, tag="res")
            nc.vector.tensor_tensor(
                out=res[:], in0=eo_ps[1][:], in1=ea0[:], op=OP.add
            )
            nc.sync.dma_start(out_d.ap(), res[:])

    nc.compile()
    _BUILT = (nc, in_names)
    return _BUILT


def make_in_maps(x, Wg, bg, W1, b1, W2, b2):
    """Host-side packing/quantization: per-core input dicts."""
    bf = ml_dtypes.bfloat16
    f8 = ml_dtypes.float8_e3m4
    x = np.asarray(x, np.float32).reshape(D)
    Wg = np.asarray(Wg, np.float32)
    bg = np.asarray(bg, np.float32).reshape(E)
    W1 = np.asarray(W1, np.float32)
    b1 = np.asarray(b1, np.float32)
    W2 = np.asarray(W2, np.float32)
    b2 = np.asarray(b2, np.float32)

    # gating pack: x_pd | WgT | bg(row 0) | partition-iota (f32 only)
    gp = np.zeros((P, 417), np.float32)
    gp[:, 0:DC] = x.reshape(DC, P).T
    gp[:, 16 : 16 + DC * E] = (
        Wg.T.reshape(DC, P, E).transpose(1, 0, 2).reshape(P, DC * E)
    )
    gp[0, 272 : 272 + E] = bg
    gp[0:E, 288] = np.arange(E, dtype=np.float32)
    gp[0, 289:417] = 1.0
    gpbf = np.ascontiguousarray(gp[:, 0:288]).astype(bf)

    xl1 = (x.reshape(DC, P).T / S1).astype(bf)
    xl1 = np.ascontiguousarray(xl1)

    w2np = bf if W2_BF16 else f8

    in_maps = []
    for c in range(NCORES):
        rs = slice(c * RS, (c + 1) * RS)
        w1q = np.ascontiguousarray(
            (W1[:, rs, :] * S1)
            .reshape(E, RS, DC, P)
            .transpose(0, 3, 2, 1)
            .reshape(E, P, DC * RS)
        ).astype(f8)
        b1pk = np.ascontiguousarray(b1[:, rs])
        # W2 slice
        w2q = np.ascontiguousarray(
            (W2[:, :, rs] * S2)
            .reshape(E, H, NCH, P)
            .transpose(0, 3, 2, 1)
            .reshape(E, P, NCH * H)
        ).astype(w2np)
        b2pk = np.ascontiguousarray(b2 * (S2 / NCORES))
        in_maps.append(
            {
                "gpbf": gpbf,
                "gpf": gp,
                "xl1": xl1,
                "w1q": w1q,
                "w2q": w2q,
                "b1pk": b1pk,
                "b2pk": b2pk,
            }
        )
    return in_maps


def kernel(x, Wg, bg, W1, b1, W2, b2, train=0, **_unused):
    from concourse import bass_utils

    nc, _ = _build()
    in_maps = make_in_maps(x, Wg, bg, W1, b1, W2, b2)
    res = bass_utils.run_bass_kernel_spmd(
        nc, in_maps, core_ids=list(range(NCORES))
    )
    outs = [
        np.asarray(res.results[c]["out"], np.float32).reshape(P, OC)
        for c in range(NCORES)
    ]
    # each core holds a gate-weighted partial over its contraction shard;
    # out[oc*128+p] = sum_c outs[c][p, oc]
    tot = np.sum(outs, axis=0, dtype=np.float32)
    return tot.T.reshape(H).copy()
